# revision 6
# baseline (speedup 1.0000x reference)
"""Multi-head self-attention (B=1, S=4096, D=2048, H=16, rotary_dim=64) on 8 TRN2 NeuronCores.

Head-sharded tensor parallelism: each core computes 2 heads end-to-end
(QKV projection + RoPE + full softmax attention) plus its slice of the
row-sharded output projection; the 8 partial [S, D] outputs are summed on
the host.

Precision: f32r (TF32-like, full PE speed) for all matmuls, fp32 softmax
denominator accumulation. Softmax skips max-subtraction (scores are
~N(0,1); exp is safe in fp32 range), so exp(s)/sum(exp(s)) is computed
directly.
"""

import numpy as np

import concourse.bass as bass
import concourse.mybir as mybir
import concourse.tile as tile
from concourse import bacc
from concourse.bass_utils import run_bass_kernel_spmd
from concourse.masks import make_identity

F32 = mybir.dt.float32
F32R = mybir.dt.float32r
BF16 = mybir.dt.bfloat16

D = 2048
H = 16
HD = 128
ROT = 64
NCORES = 8
HPC = H // NCORES  # heads per core
SCALE = float(HD) ** -0.5

_CACHE = {}


def build_module(S=4096, ST=256, QTL=512):
    """Build the per-core SPMD bass module. Returns compiled nc."""
    NST = S // ST        # phase-1 s-tiles
    NKT = D // 128       # contraction tiles for QKV
    NQT = S // QTL       # phase-2 q-tiles
    NKC = S // 128       # attention k-chunks
    NKP = NKC // 2       # attention k-chunk pairs
    NSC = S // 128       # out-proj s-chunks
    ETL = 512            # out-proj e-tile
    NET = D // ETL

    nc = bacc.Bacc(None, target_bir_lowering=False, debug=True)

    xT_d = nc.dram_tensor("xT", [D, S], F32R, kind="ExternalInput")
    w_d = nc.dram_tensor("wsl", [D, 3 * HPC, 128], F32R, kind="ExternalInput")
    wo_d = nc.dram_tensor("wout", [HPC * HD, D], F32R, kind="ExternalInput")
    b_d = nc.dram_tensor("bsl", [128, 3 * HPC], F32, kind="ExternalInput")
    cs_d = nc.dram_tensor("cs", [ROT, 2, S], F32R, kind="ExternalInput")
    y_d = nc.dram_tensor("y", [S, D], F32, kind="ExternalOutput")

    xT_r = xT_d[:].rearrange("(t p) s -> p t s", p=128)
    w_r = w_d[:].rearrange("(t p) j m -> p t j m", p=128)
    wo_r = wo_d[:].rearrange("(t p) e -> p t e", p=128)

    with tile.TileContext(nc) as tc:
        with tc.tile_pool(name="persist", bufs=1) as P:
            # Persistent per-head tensors
            QT = [P.tile([128, S], F32R, tag=f"qt{h}", name=f"qt{h}") for h in range(HPC)]
            KT = [P.tile([128, S], F32R, tag=f"kt{h}", name=f"kt{h}") for h in range(HPC)]
            V = [P.tile([128, NKC, 128], F32R, tag=f"v{h}", name=f"v{h}") for h in range(HPC)]
            b_sb = P.tile([128, 3 * HPC], F32)
            ident_f = P.tile([128, 128], F32)
            identr = P.tile([128, 128], F32R)
            ones_f = P.tile([128, 128], F32)
            ones_r = P.tile([128, 128], F32R)
            nc.sync.dma_start(b_sb[:], b_d[:])
            make_identity(nc, ident_f)
            nc.vector.tensor_copy(identr[:], ident_f[:])
            nc.vector.memset(ones_f[:], 1.0)
            nc.vector.tensor_copy(ones_r[:], ones_f[:])

            # ---------- Phase 1: QKV projection + RoPE + V transpose ----------
            with (
                tc.tile_pool(name="wp", bufs=1) as wp,
                tc.tile_pool(name="xp", bufs=2) as xp,
                tc.tile_pool(name="csp", bufs=2) as csp,
                tc.tile_pool(name="vtp", bufs=2) as vtp,
                tc.tile_pool(name="rtp", bufs=2) as rtp,
                tc.tile_pool(name="ps1", bufs=2, space="PSUM") as ps1,
                tc.tile_pool(name="pst", bufs=2, space="PSUM") as pst,
            ):
                w_sb = wp.tile([128, NKT, 3 * HPC, 128], F32R)
                nc.sync.dma_start(w_sb[:], w_r)

                for st in range(NST):
                    sl = bass.ts(st, ST)
                    xt = xp.tile([128, NKT, ST], F32R, tag="xt")
                    nc.sync.dma_start(xt[:], xT_r[:, :, sl])
                    cst = csp.tile([ROT, 2, ST], F32R, tag="cst")
                    nc.sync.dma_start(cst[:], cs_d[:, :, sl])

                    for h in range(HPC):
                        j = 3 * h
                        # Q^T and K^T chunks
                        for which, dst in ((0, QT[h]), (1, KT[h])):
                            ps = ps1.tile([128, ST], F32, tag="psqk")
                            for k in range(NKT):
                                nc.tensor.matmul(
                                    ps[:], w_sb[:, k, j + which, :], xt[:, k, :],
                                    start=(k == 0), stop=(k == NKT - 1),
                                )
                            nc.scalar.activation(
                                dst[:, sl], ps[:], mybir.ActivationFunctionType.Identity,
                                bias=b_sb[:, j + which : j + which + 1],
                            )
                        # V^T chunk -> PE-transpose to V[k, d]
                        psv = ps1.tile([128, ST], F32, tag="psv")
                        for k in range(NKT):
                            nc.tensor.matmul(
                                psv[:], w_sb[:, k, j + 2, :], xt[:, k, :],
                                start=(k == 0), stop=(k == NKT - 1),
                            )
                        vt = vtp.tile([128, ST], F32R, tag="vt")
                        nc.scalar.activation(
                            vt[:], psv[:], mybir.ActivationFunctionType.Identity,
                            bias=b_sb[:, j + 2 : j + 3],
                        )
                        for sc in range(ST // 128):
                            ptr = pst.tile([128, 128], F32R, tag="ptr")
                            nc.tensor.transpose(ptr[:], vt[:, bass.ts(sc, 128)], identr[:])
                            nc.vector.tensor_copy(V[h][:, st * (ST // 128) + sc, :], ptr[:])

                        # RoPE on rotary partitions of this s-slice
                        for T in (QT[h], KT[h]):
                            tmp = rtp.tile([ROT, ST], F32R, tag="rtmp")
                            nc.vector.tensor_copy(tmp[0 : ROT // 2, :], T[ROT // 2 : ROT, sl])
                            nc.vector.tensor_copy(tmp[ROT // 2 : ROT, :], T[0 : ROT // 2, sl])
                            nc.vector.tensor_mul(tmp[:], tmp[:], cst[:, 1, :])
                            nc.vector.tensor_mul(T[0:ROT, sl], T[0:ROT, sl], cst[:, 0, :])
                            nc.vector.tensor_add(T[0:ROT, sl], T[0:ROT, sl], tmp[:])

            # ---------- Phase 2+3: attention with folded output projection ----------
            with tc.tile_pool(name="atp", bufs=1) as atp:
                AT = [atp.tile([128, S], F32R, tag=f"at{h}", name=f"at{h}") for h in range(HPC)]
                with (
                    tc.tile_pool(name="wop", bufs=1) as wop,
                    tc.tile_pool(name="ptp", bufs=3) as ptp,
                    tc.tile_pool(name="accAp", bufs=2) as accAp,
                    tc.tile_pool(name="accBp", bufs=2) as accBp,
                    tc.tile_pool(name="rcp", bufs=2) as rcp,
                    tc.tile_pool(name="yp", bufs=3) as yp,
                    tc.tile_pool(name="pss", bufs=2, space="PSUM") as pssp,
                    tc.tile_pool(name="pso", bufs=1, space="PSUM") as psop,
                    tc.tile_pool(name="psd", bufs=1, space="PSUM") as psdp,
                    tc.tile_pool(name="psy", bufs=2, space="PSUM") as psyp,
                ):
                    wo_sb = wop.tile([128, HPC, D], F32R)
                    nc.sync.dma_start(wo_sb[:], wo_r)
                    for qt in range(NQT):
                        qsl = bass.ts(qt, QTL)
                        for h in range(HPC):
                            oacc = psop.tile([128, QTL], F32, tag="oacc")
                            accA = accAp.tile([128, 2, QTL], F32R, tag="accA")
                            accB = accBp.tile([128, 2, QTL], F32R, tag="accB")
                            for kp in range(NKP):
                                pss = pssp.tile([128, 2, QTL], F32, tag="pss")
                                for jj in (0, 1):
                                    kc = 2 * kp + jj
                                    nc.tensor.matmul(
                                        pss[:, jj, :], KT[h][:, bass.ts(kc, 128)], QT[h][:, qsl],
                                        start=True, stop=True,
                                    )
                                pt = ptp.tile([128, 2, QTL], F32R, tag="pt")
                                nc.scalar.activation(
                                    pt[:], pss[:], mybir.ActivationFunctionType.Exp,
                                    scale=SCALE,
                                )
                                for jj in (0, 1):
                                    kc = 2 * kp + jj
                                    nc.tensor.matmul(
                                        oacc[:], V[h][:, kc, :], pt[:, jj, :],
                                        start=(kc == 0), stop=(kc == NKC - 1),
                                    )
                                # denominator partial sums, split DVE / GpSimd
                                if kp == 0:
                                    nc.vector.tensor_copy(accA[:], pt[:])
                                elif kp == 1:
                                    nc.gpsimd.tensor_copy(accB[:], pt[:])
                                elif kp % 2 == 0:
                                    nc.vector.tensor_add(accA[:], accA[:], pt[:])
                                else:
                                    nc.gpsimd.tensor_add(accB[:], accB[:], pt[:])
                            # denominator: all-ones matmul -> broadcast across partitions
                            dn = psdp.tile([128, QTL], F32, tag="dn")
                            nc.tensor.matmul(dn[:], ones_r[:], accA[:, 0, :], start=True, stop=False)
                            nc.tensor.matmul(dn[:], ones_r[:], accA[:, 1, :], start=False, stop=False)
                            nc.tensor.matmul(dn[:], ones_r[:], accB[:, 0, :], start=False, stop=False)
                            nc.tensor.matmul(dn[:], ones_r[:], accB[:, 1, :], start=False, stop=True)
                            rc = rcp.tile([128, QTL], F32, tag="rc")
                            scr = rcp.tile([128, QTL], F32, tag="rcscr")
                            nc.vector.reciprocal_approx_accurate(rc[:], dn[:], scr[:])
                            nc.vector.tensor_mul(AT[h][:, qsl], oacc[:], rc[:])
                        # output projection for this qt's s-chunks (both heads ready)
                        for sc4 in range(QTL // 128):
                            ssl = bass.ds(qt * QTL + sc4 * 128, 128)
                            for et in range(NET):
                                esl = bass.ts(et, ETL)
                                psy = psyp.tile([128, ETL], F32, tag="psy")
                                for h in range(HPC):
                                    nc.tensor.matmul(
                                        psy[:], AT[h][:, ssl], wo_sb[:, h, esl],
                                        start=(h == 0), stop=(h == HPC - 1),
                                    )
                                yt = yp.tile([128, ETL], F32, tag="yt")
                                nc.vector.tensor_copy(yt[:], psy[:])
                                nc.sync.dma_start(y_d[ssl, esl], yt[:])

    nc.compile()
    return nc


def _host_prep(x, w_qkv, b_qkv, w_out, S):
    """Build per-core input maps."""
    xT = np.ascontiguousarray(x.reshape(S, D).T).astype(np.float32)

    # RoPE tables (match reference._rope_cos_sin)
    inv_freq = (1.0 / (10000.0 ** (np.arange(0, ROT, 2, dtype=np.float32) / ROT))).astype(np.float32)
    t = np.arange(S, dtype=np.float32)
    freqs = np.outer(t, inv_freq)                      # [S, ROT/2]
    emb = np.concatenate([freqs, freqs], axis=-1)      # [S, ROT]
    cosT = np.cos(emb).astype(np.float32).T            # [ROT, S]
    sinT = np.sin(emb).astype(np.float32).T
    sinS = sinT.copy()
    sinS[0 : ROT // 2] *= -1.0
    cs = np.ascontiguousarray(np.stack([cosT, sinS], axis=1))  # [ROT, 2, S]

    in_maps = []
    for c in range(NCORES):
        cols = []
        bcols = []
        for h in [HPC * c + i for i in range(HPC)]:
            for part in range(3):  # q, k, v
                off = part * D + h * HD
                cols.append(w_qkv[:, off : off + HD])
                bcols.append(b_qkv[off : off + HD])
        wsl = np.ascontiguousarray(np.stack(cols, axis=1)).astype(np.float32)   # [D, 3*HPC, 128]
        bsl = np.ascontiguousarray(np.stack(bcols, axis=1)).astype(np.float32)  # [128, 3*HPC]
        wout_sl = np.ascontiguousarray(w_out[c * HPC * HD : (c + 1) * HPC * HD, :]).astype(np.float32)
        in_maps.append({"xT": xT, "wsl": wsl, "bsl": bsl, "wout": wout_sl, "cs": cs})
    return in_maps


def kernel(x, w_qkv, b_qkv, w_out, b_out):
    B, S, D_ = x.shape
    assert B == 1 and D_ == D
    if "nc" not in _CACHE:
        _CACHE["nc"] = build_module(S=S)
    nc = _CACHE["nc"]
    in_maps = _host_prep(np.asarray(x, dtype=np.float32), np.asarray(w_qkv, dtype=np.float32),
                         np.asarray(b_qkv, dtype=np.float32), np.asarray(w_out, dtype=np.float32), S)
    res = run_bass_kernel_spmd(nc, in_maps, list(range(NCORES)))
    y = np.zeros((S, D), dtype=np.float32)
    for c in range(NCORES):
        y += res.results[c]["y"]
    y += np.asarray(b_out, dtype=np.float32)[None, :]
    return y.reshape(1, S, D)


# revision 7
# speedup vs baseline: 1.0350x; 1.0350x over previous
"""Multi-head self-attention (B=1, S=4096, D=2048, H=16, rotary_dim=64) on 8 TRN2 NeuronCores.

Head-sharded tensor parallelism: each core computes 2 heads end-to-end
(QKV projection + RoPE + full softmax attention) plus its slice of the
row-sharded output projection; the 8 partial [S, D] outputs are summed on
the host.

Precision: f32r (TF32-like, full PE speed) for all matmuls, fp32 softmax
denominator accumulation. Softmax skips max-subtraction (scores are
~N(0,1); exp is safe in fp32 range), so exp(s)/sum(exp(s)) is computed
directly.
"""

import numpy as np

import concourse.bass as bass
import concourse.mybir as mybir
import concourse.tile as tile
from concourse import bacc
from concourse.bass_utils import run_bass_kernel_spmd
from concourse.masks import make_identity

F32 = mybir.dt.float32
F32R = mybir.dt.float32r
BF16 = mybir.dt.bfloat16

D = 2048
H = 16
HD = 128
ROT = 64
NCORES = 8
HPC = H // NCORES  # heads per core
SCALE = float(HD) ** -0.5

_CACHE = {}


def build_module(S=4096, ST=512, QTL=512):
    """Build the per-core SPMD bass module. Returns compiled nc."""
    NST = S // ST        # phase-1 s-tiles
    NKT = D // 128       # contraction tiles for QKV
    NQT = S // QTL       # phase-2 q-tiles
    NKC = S // 128       # attention k-chunks
    NKP = NKC // 2       # attention k-chunk pairs
    NSC = S // 128       # out-proj s-chunks
    ETL = 512            # out-proj e-tile
    NET = D // ETL

    nc = bacc.Bacc(None, target_bir_lowering=False, debug=True)

    xT_d = nc.dram_tensor("xT", [D, S], F32R, kind="ExternalInput")
    w_d = nc.dram_tensor("wsl", [D, 3 * HPC, 128], F32R, kind="ExternalInput")
    wo_d = nc.dram_tensor("wout", [HPC * HD, D], F32R, kind="ExternalInput")
    b_d = nc.dram_tensor("bsl", [128, 3 * HPC], F32, kind="ExternalInput")
    cs_d = nc.dram_tensor("cs", [ROT, 2, S], F32R, kind="ExternalInput")
    y_d = nc.dram_tensor("y", [S, D], F32, kind="ExternalOutput")

    xT_r = xT_d[:].rearrange("(t p) s -> p t s", p=128)
    w_r = w_d[:].rearrange("(t p) j m -> p t j m", p=128)
    wo_r = wo_d[:].rearrange("(t p) e -> p t e", p=128)

    with tile.TileContext(nc) as tc:
        with tc.tile_pool(name="persist", bufs=1) as P:
            # Persistent per-head tensors
            QT = [P.tile([128, S], F32R, tag=f"qt{h}", name=f"qt{h}") for h in range(HPC)]
            KT = [P.tile([128, S], F32R, tag=f"kt{h}", name=f"kt{h}") for h in range(HPC)]
            V = [P.tile([128, NKC, 128], F32R, tag=f"v{h}", name=f"v{h}") for h in range(HPC)]
            b_sb = P.tile([128, 3 * HPC], F32)
            ident_f = P.tile([128, 128], F32)
            identr = P.tile([128, 128], F32R)
            ones_f = P.tile([128, 128], F32)
            ones_r = P.tile([128, 128], F32R)
            nc.sync.dma_start(b_sb[:], b_d[:])
            make_identity(nc, ident_f)
            nc.vector.tensor_copy(identr[:], ident_f[:])
            nc.vector.memset(ones_f[:], 1.0)
            nc.vector.tensor_copy(ones_r[:], ones_f[:])

            # ---------- Phase 1: QKV projection + RoPE + V transpose ----------
            with (
                tc.tile_pool(name="wp", bufs=1) as wp,
                tc.tile_pool(name="xp", bufs=2) as xp,
                tc.tile_pool(name="csp", bufs=2) as csp,
                tc.tile_pool(name="vtp", bufs=2) as vtp,
                tc.tile_pool(name="rtp", bufs=2) as rtp,
                tc.tile_pool(name="ps1", bufs=1, space="PSUM") as ps1,
                tc.tile_pool(name="pst", bufs=2, space="PSUM") as pst,
            ):
                w_sb = wp.tile([128, NKT, 3 * HPC, 128], F32R)
                nc.sync.dma_start(w_sb[:], w_r)

                KH = NKT // 2
                for st in range(NST):
                    sl = bass.ts(st, ST)
                    cst = csp.tile([ROT, 2, ST], F32R, tag="cst")
                    nc.sync.dma_start(cst[:], cs_d[:, :, sl])

                    pss_qkv = {}
                    for h in range(HPC):
                        pss_qkv[(h, 0)] = ps1.tile([128, ST], F32, tag=f"psq{h}", name=f"psq{h}_{st}")
                        pss_qkv[(h, 1)] = ps1.tile([128, ST], F32, tag=f"psk{h}", name=f"psk{h}_{st}")
                        pss_qkv[(h, 2)] = ps1.tile([128, ST], F32, tag=f"psv{h}", name=f"psv{h}_{st}")
                    for kh in range(2):
                        xt = xp.tile([128, KH, ST], F32R, tag="xt")
                        nc.sync.dma_start(xt[:], xT_r[:, kh * KH : (kh + 1) * KH, sl])
                        for h in range(HPC):
                            j = 3 * h
                            for which in range(3):
                                ps = pss_qkv[(h, which)]
                                for k in range(KH):
                                    nc.tensor.matmul(
                                        ps[:], w_sb[:, kh * KH + k, j + which, :], xt[:, k, :],
                                        start=(kh == 0 and k == 0), stop=(kh == 1 and k == KH - 1),
                                    )
                    for h in range(HPC):
                        j = 3 * h
                        for which, dst in ((0, QT[h]), (1, KT[h])):
                            nc.scalar.activation(
                                dst[:, sl], pss_qkv[(h, which)][:],
                                mybir.ActivationFunctionType.Identity,
                                bias=b_sb[:, j + which : j + which + 1],
                            )
                        vt = vtp.tile([128, ST], F32R, tag="vt")
                        nc.scalar.activation(
                            vt[:], pss_qkv[(h, 2)][:], mybir.ActivationFunctionType.Identity,
                            bias=b_sb[:, j + 2 : j + 3],
                        )
                        for sc in range(ST // 128):
                            ptr = pst.tile([128, 128], F32R, tag="ptr")
                            nc.tensor.transpose(ptr[:], vt[:, bass.ts(sc, 128)], identr[:])
                            nc.scalar.activation(V[h][:, st * (ST // 128) + sc, :], ptr[:],
                                                 mybir.ActivationFunctionType.Copy)

                        # RoPE on rotary partitions of this s-slice
                        for T in (QT[h], KT[h]):
                            tmp = rtp.tile([ROT, ST], F32R, tag="rtmp")
                            nc.vector.tensor_copy(tmp[0 : ROT // 2, :], T[ROT // 2 : ROT, sl])
                            nc.vector.tensor_copy(tmp[ROT // 2 : ROT, :], T[0 : ROT // 2, sl])
                            nc.vector.tensor_mul(tmp[:], tmp[:], cst[:, 1, :])
                            nc.vector.tensor_mul(T[0:ROT, sl], T[0:ROT, sl], cst[:, 0, :])
                            nc.vector.tensor_add(T[0:ROT, sl], T[0:ROT, sl], tmp[:])

            # ---------- Phase 2+3: attention with folded output projection ----------
            with tc.tile_pool(name="atp", bufs=1) as atp:
                AT = [atp.tile([128, S], F32R, tag=f"at{h}", name=f"at{h}") for h in range(HPC)]
                with (
                    tc.tile_pool(name="wop", bufs=1) as wop,
                    tc.tile_pool(name="ptp", bufs=3) as ptp,
                    tc.tile_pool(name="accAp", bufs=2) as accAp,
                    tc.tile_pool(name="accBp", bufs=2) as accBp,
                    tc.tile_pool(name="rcp", bufs=2) as rcp,
                    tc.tile_pool(name="yp", bufs=3) as yp,
                    tc.tile_pool(name="pss", bufs=2, space="PSUM") as pssp,
                    tc.tile_pool(name="pso", bufs=1, space="PSUM") as psop,
                    tc.tile_pool(name="psd", bufs=1, space="PSUM") as psdp,
                    tc.tile_pool(name="psy", bufs=2, space="PSUM") as psyp,
                ):
                    wo_sb = wop.tile([128, HPC, D], F32R)
                    nc.sync.dma_start(wo_sb[:], wo_r)
                    for qt in range(NQT):
                        qsl = bass.ts(qt, QTL)
                        for h in range(HPC):
                            oacc = psop.tile([128, QTL], F32, tag="oacc")
                            accA = accAp.tile([128, 2, QTL], F32R, tag="accA")
                            accB = accBp.tile([128, 2, QTL], F32R, tag="accB")
                            for kp in range(NKP):
                                pss = pssp.tile([128, 2, QTL], F32, tag="pss")
                                for jj in (0, 1):
                                    kc = 2 * kp + jj
                                    nc.tensor.matmul(
                                        pss[:, jj, :], KT[h][:, bass.ts(kc, 128)], QT[h][:, qsl],
                                        start=True, stop=True,
                                    )
                                pt = ptp.tile([128, 2, QTL], F32R, tag="pt")
                                nc.scalar.activation(
                                    pt[:], pss[:], mybir.ActivationFunctionType.Exp,
                                    scale=SCALE,
                                )
                                for jj in (0, 1):
                                    kc = 2 * kp + jj
                                    nc.tensor.matmul(
                                        oacc[:], V[h][:, kc, :], pt[:, jj, :],
                                        start=(kc == 0), stop=(kc == NKC - 1),
                                    )
                                # denominator partial sums, split DVE / GpSimd (~2:1)
                                if kp == 0:
                                    nc.vector.tensor_copy(accA[:], pt[:])
                                elif kp == 1:
                                    nc.gpsimd.tensor_copy(accB[:], pt[:])
                                elif kp % 3 == 1:
                                    nc.gpsimd.tensor_add(accB[:], accB[:], pt[:])
                                else:
                                    nc.vector.tensor_add(accA[:], accA[:], pt[:])
                            # denominator: all-ones matmul -> broadcast across partitions
                            dn = psdp.tile([128, QTL], F32, tag="dn")
                            nc.tensor.matmul(dn[:], ones_r[:], accA[:, 0, :], start=True, stop=False)
                            nc.tensor.matmul(dn[:], ones_r[:], accA[:, 1, :], start=False, stop=False)
                            nc.tensor.matmul(dn[:], ones_r[:], accB[:, 0, :], start=False, stop=False)
                            nc.tensor.matmul(dn[:], ones_r[:], accB[:, 1, :], start=False, stop=True)
                            rc = rcp.tile([128, QTL], F32, tag="rc")
                            scr = rcp.tile([128, QTL], F32, tag="rcscr")
                            nc.vector.reciprocal_approx_accurate(rc[:], dn[:], scr[:])
                            nc.vector.tensor_mul(AT[h][:, qsl], oacc[:], rc[:])
                        # output projection for this qt's s-chunks (both heads ready)
                        for sc4 in range(QTL // 128):
                            ssl = bass.ds(qt * QTL + sc4 * 128, 128)
                            for et in range(NET):
                                esl = bass.ts(et, ETL)
                                psy = psyp.tile([128, ETL], F32, tag="psy")
                                for h in range(HPC):
                                    nc.tensor.matmul(
                                        psy[:], AT[h][:, ssl], wo_sb[:, h, esl],
                                        start=(h == 0), stop=(h == HPC - 1),
                                    )
                                yt = yp.tile([128, ETL], F32, tag="yt")
                                if et % 2 == 0:
                                    nc.vector.tensor_copy(yt[:], psy[:])
                                else:
                                    nc.scalar.activation(yt[:], psy[:],
                                                         mybir.ActivationFunctionType.Copy)
                                nc.sync.dma_start(y_d[ssl, esl], yt[:])

    nc.compile()
    return nc


def _host_prep(x, w_qkv, b_qkv, w_out, S):
    """Build per-core input maps."""
    xT = np.ascontiguousarray(x.reshape(S, D).T).astype(np.float32)

    # RoPE tables (match reference._rope_cos_sin)
    inv_freq = (1.0 / (10000.0 ** (np.arange(0, ROT, 2, dtype=np.float32) / ROT))).astype(np.float32)
    t = np.arange(S, dtype=np.float32)
    freqs = np.outer(t, inv_freq)                      # [S, ROT/2]
    emb = np.concatenate([freqs, freqs], axis=-1)      # [S, ROT]
    cosT = np.cos(emb).astype(np.float32).T            # [ROT, S]
    sinT = np.sin(emb).astype(np.float32).T
    sinS = sinT.copy()
    sinS[0 : ROT // 2] *= -1.0
    cs = np.ascontiguousarray(np.stack([cosT, sinS], axis=1))  # [ROT, 2, S]

    in_maps = []
    for c in range(NCORES):
        cols = []
        bcols = []
        for h in [HPC * c + i for i in range(HPC)]:
            for part in range(3):  # q, k, v
                off = part * D + h * HD
                cols.append(w_qkv[:, off : off + HD])
                bcols.append(b_qkv[off : off + HD])
        wsl = np.ascontiguousarray(np.stack(cols, axis=1)).astype(np.float32)   # [D, 3*HPC, 128]
        bsl = np.ascontiguousarray(np.stack(bcols, axis=1)).astype(np.float32)  # [128, 3*HPC]
        wout_sl = np.ascontiguousarray(w_out[c * HPC * HD : (c + 1) * HPC * HD, :]).astype(np.float32)
        in_maps.append({"xT": xT, "wsl": wsl, "bsl": bsl, "wout": wout_sl, "cs": cs})
    return in_maps


def kernel(x, w_qkv, b_qkv, w_out, b_out):
    B, S, D_ = x.shape
    assert B == 1 and D_ == D
    if "nc" not in _CACHE:
        _CACHE["nc"] = build_module(S=S)
    nc = _CACHE["nc"]
    in_maps = _host_prep(np.asarray(x, dtype=np.float32), np.asarray(w_qkv, dtype=np.float32),
                         np.asarray(b_qkv, dtype=np.float32), np.asarray(w_out, dtype=np.float32), S)
    res = run_bass_kernel_spmd(nc, in_maps, list(range(NCORES)))
    y = np.zeros((S, D), dtype=np.float32)
    for c in range(NCORES):
        y += res.results[c]["y"]
    y += np.asarray(b_out, dtype=np.float32)[None, :]
    return y.reshape(1, S, D)


# revision 8
# speedup vs baseline: 1.0919x; 1.0549x over previous
"""Multi-head self-attention (B=1, S=4096, D=2048, H=16, rotary_dim=64) on 8 TRN2 NeuronCores.

Head-sharded tensor parallelism: each core computes 2 heads end-to-end
(QKV projection + RoPE + full softmax attention) plus its slice of the
row-sharded output projection; the 8 partial [S, D] outputs are summed on
the host.

Precision: f32r (TF32-like, full PE speed) for all matmuls, fp32 softmax
denominator accumulation. Softmax skips max-subtraction (scores are
~N(0,1); exp is safe in fp32 range), so exp(s)/sum(exp(s)) is computed
directly.
"""

import numpy as np

import concourse.bass as bass
import concourse.mybir as mybir
import concourse.tile as tile
from concourse import bacc
from concourse.bass_utils import run_bass_kernel_spmd
from concourse.masks import make_identity

F32 = mybir.dt.float32
F32R = mybir.dt.float32r
BF16 = mybir.dt.bfloat16

D = 2048
H = 16
HD = 128
ROT = 64
NCORES = 8
HPC = H // NCORES  # heads per core
SCALE = float(HD) ** -0.5

_CACHE = {}


def build_module(S=4096, ST=512, QTL=512):
    """Build the per-core SPMD bass module. Returns compiled nc."""
    NST = S // ST        # phase-1 s-tiles
    NKT = D // 128       # contraction tiles for QKV
    NQT = S // QTL       # phase-2 q-tiles
    NKC = S // 128       # attention k-chunks
    NKP = NKC // 2       # attention k-chunk pairs
    NSC = S // 128       # out-proj s-chunks
    ETL = 512            # out-proj e-tile
    NET = D // ETL

    nc = bacc.Bacc(None, target_bir_lowering=False, debug=True)

    xT_d = nc.dram_tensor("xT", [D, S], F32R, kind="ExternalInput")
    w_d = nc.dram_tensor("wsl", [D, 3 * HPC, 128], F32R, kind="ExternalInput")
    wo_d = nc.dram_tensor("wout", [HPC * HD, D], F32R, kind="ExternalInput")
    b_d = nc.dram_tensor("bsl", [128, 3 * HPC], F32, kind="ExternalInput")
    cs_d = nc.dram_tensor("cs", [ROT, 2, S], F32R, kind="ExternalInput")
    y_d = nc.dram_tensor("y", [S, D], F32, kind="ExternalOutput")

    xT_r = xT_d[:].rearrange("(t p) s -> p t s", p=128)
    w_r = w_d[:].rearrange("(t p) j m -> p t j m", p=128)
    wo_r = wo_d[:].rearrange("(t p) e -> p t e", p=128)

    with tile.TileContext(nc) as tc:
        with tc.tile_pool(name="persist", bufs=1) as P:
            # Persistent per-head tensors
            QT = [P.tile([128, S], F32R, tag=f"qt{h}", name=f"qt{h}") for h in range(HPC)]
            KT = [P.tile([128, S], F32R, tag=f"kt{h}", name=f"kt{h}") for h in range(HPC)]
            V = [P.tile([128, NKC, 128], F32R, tag=f"v{h}", name=f"v{h}") for h in range(HPC)]
            b_sb = P.tile([128, 3 * HPC], F32)
            ident_f = P.tile([128, 128], F32)
            identr = P.tile([128, 128], F32R)
            ones_f = P.tile([128, 128], F32)
            ones_r = P.tile([128, 128], F32R)
            nc.sync.dma_start(b_sb[:], b_d[:])
            make_identity(nc, ident_f)
            nc.vector.tensor_copy(identr[:], ident_f[:])
            nc.vector.memset(ones_f[:], 1.0)
            nc.vector.tensor_copy(ones_r[:], ones_f[:])

            # ---------- Phase 1: QKV projection + RoPE + V transpose ----------
            with (
                tc.tile_pool(name="wp", bufs=1) as wp,
                tc.tile_pool(name="xp", bufs=2) as xp,
                tc.tile_pool(name="csp", bufs=2) as csp,
                tc.tile_pool(name="vtp", bufs=2) as vtp,
                tc.tile_pool(name="rtp", bufs=2) as rtp,
                tc.tile_pool(name="ps1", bufs=1, space="PSUM") as ps1,
                tc.tile_pool(name="pst", bufs=2, space="PSUM") as pst,
            ):
                w_sb = wp.tile([128, NKT, 3 * HPC, 128], F32R)
                nc.sync.dma_start(w_sb[:, 0 : NKT // 2, :, :], w_r[:, 0 : NKT // 2, :, :])
                nc.sync.dma_start(w_sb[:, NKT // 2 :, :, :], w_r[:, NKT // 2 :, :, :])

                KH = NKT // 2
                for st in range(NST):
                    sl = bass.ts(st, ST)
                    cst = csp.tile([ROT, 2, ST], F32R, tag="cst")
                    nc.sync.dma_start(cst[:], cs_d[:, :, sl])

                    pss_qkv = {}
                    for h in range(HPC):
                        pss_qkv[(h, 0)] = ps1.tile([128, ST], F32, tag=f"psq{h}", name=f"psq{h}_{st}")
                        pss_qkv[(h, 1)] = ps1.tile([128, ST], F32, tag=f"psk{h}", name=f"psk{h}_{st}")
                        pss_qkv[(h, 2)] = ps1.tile([128, ST], F32, tag=f"psv{h}", name=f"psv{h}_{st}")
                    for kh in range(2):
                        xt = xp.tile([128, KH, ST], F32R, tag="xt")
                        nc.sync.dma_start(xt[:], xT_r[:, kh * KH : (kh + 1) * KH, sl])
                        for h in range(HPC):
                            j = 3 * h
                            for which in range(3):
                                ps = pss_qkv[(h, which)]
                                for k in range(KH):
                                    nc.tensor.matmul(
                                        ps[:], w_sb[:, kh * KH + k, j + which, :], xt[:, k, :],
                                        start=(kh == 0 and k == 0), stop=(kh == 1 and k == KH - 1),
                                    )
                    for h in range(HPC):
                        j = 3 * h
                        for which, dst in ((0, QT[h]), (1, KT[h])):
                            nc.scalar.activation(
                                dst[:, sl], pss_qkv[(h, which)][:],
                                mybir.ActivationFunctionType.Identity,
                                bias=b_sb[:, j + which : j + which + 1],
                            )
                        vt = vtp.tile([128, ST], F32R, tag="vt")
                        nc.scalar.activation(
                            vt[:], pss_qkv[(h, 2)][:], mybir.ActivationFunctionType.Identity,
                            bias=b_sb[:, j + 2 : j + 3],
                        )
                        for sc in range(ST // 128):
                            ptr = pst.tile([128, 128], F32R, tag="ptr")
                            nc.tensor.transpose(ptr[:], vt[:, bass.ts(sc, 128)], identr[:])
                            nc.scalar.activation(V[h][:, st * (ST // 128) + sc, :], ptr[:],
                                                 mybir.ActivationFunctionType.Copy)

                        # RoPE on rotary partitions of this s-slice
                        for T in (QT[h], KT[h]):
                            tmp = rtp.tile([ROT, ST], F32R, tag="rtmp")
                            nc.vector.tensor_copy(tmp[0 : ROT // 2, :], T[ROT // 2 : ROT, sl])
                            nc.vector.tensor_copy(tmp[ROT // 2 : ROT, :], T[0 : ROT // 2, sl])
                            nc.vector.tensor_mul(tmp[:], tmp[:], cst[:, 1, :])
                            nc.vector.tensor_mul(T[0:ROT, sl], T[0:ROT, sl], cst[:, 0, :])
                            nc.vector.tensor_add(T[0:ROT, sl], T[0:ROT, sl], tmp[:])

            # ---------- Phase 2+3: attention with folded output projection ----------
            with tc.tile_pool(name="atp", bufs=1) as atp:
                AT = [atp.tile([128, S], F32R, tag=f"at{h}", name=f"at{h}") for h in range(HPC)]
                with (
                    tc.tile_pool(name="wop", bufs=1) as wop,
                    tc.tile_pool(name="ptp", bufs=4) as ptp,
                    tc.tile_pool(name="accAp", bufs=2) as accAp,
                    tc.tile_pool(name="accBp", bufs=2) as accBp,
                    tc.tile_pool(name="rcp", bufs=2) as rcp,
                    tc.tile_pool(name="yp", bufs=3) as yp,
                    tc.tile_pool(name="pss", bufs=3, space="PSUM") as pssp,
                    tc.tile_pool(name="pso", bufs=2, space="PSUM") as psop,
                    tc.tile_pool(name="psd", bufs=1, space="PSUM") as psdp,
                    tc.tile_pool(name="psy", bufs=2, space="PSUM") as psyp,
                ):
                    wo_sb = wop.tile([128, HPC, D], F32R)
                    nc.sync.dma_start(wo_sb[:], wo_r)
                    LAG = 3
                    for qt in range(NQT):
                        qsl = bass.ts(qt, QTL)
                        for h in range(HPC):
                            oacc = psop.tile([128, QTL], F32, tag="oacc")
                            accA = accAp.tile([128, 2, QTL], F32R, tag="accA")
                            accB = accBp.tile([128, 2, QTL], F32R, tag="accB")
                            pts = {}
                            for kc in range(NKC + LAG):
                                if kc < NKC:
                                    # scores + exp for chunk kc
                                    pss = pssp.tile([128, QTL], F32, tag="pss",
                                                    name=f"pss_{qt}_{h}_{kc}")
                                    nc.tensor.matmul(
                                        pss[:], KT[h][:, bass.ts(kc, 128)], QT[h][:, qsl],
                                        start=True, stop=True,
                                    )
                                    kp = kc // 2
                                    if kc % 2 == 0:
                                        pts[kp] = ptp.tile([128, 2, QTL], F32R, tag="pt",
                                                           name=f"pt_{qt}_{h}_{kp}")
                                    nc.scalar.activation(
                                        pts[kp][:, kc % 2, :], pss[:],
                                        mybir.ActivationFunctionType.Exp, scale=SCALE,
                                    )
                                kd = kc - LAG
                                if 0 <= kd < NKC:
                                    # AV accumulate for chunk kd (lagged for pipelining)
                                    kp = kd // 2
                                    nc.tensor.matmul(
                                        oacc[:], V[h][:, kd, :], pts[kp][:, kd % 2, :],
                                        start=(kd == 0), stop=(kd == NKC - 1),
                                    )
                                    if kd % 2 == 1:
                                        # denominator partial sums, split DVE / GpSimd (~2:1)
                                        if kp == 0:
                                            nc.vector.tensor_copy(accA[:], pts[kp][:])
                                        elif kp == 1:
                                            nc.gpsimd.tensor_copy(accB[:], pts[kp][:])
                                        elif kp % 3 == 1:
                                            nc.gpsimd.tensor_add(accB[:], accB[:], pts[kp][:])
                                        else:
                                            nc.vector.tensor_add(accA[:], accA[:], pts[kp][:])
                                        del pts[kp]
                            # denominator: all-ones matmul -> broadcast across partitions
                            dn = psdp.tile([128, QTL], F32, tag="dn")
                            nc.tensor.matmul(dn[:], ones_r[:], accA[:, 0, :], start=True, stop=False)
                            nc.tensor.matmul(dn[:], ones_r[:], accA[:, 1, :], start=False, stop=False)
                            nc.tensor.matmul(dn[:], ones_r[:], accB[:, 0, :], start=False, stop=False)
                            nc.tensor.matmul(dn[:], ones_r[:], accB[:, 1, :], start=False, stop=True)
                            rc = rcp.tile([128, QTL], F32, tag="rc")
                            scr = rcp.tile([128, QTL], F32, tag="rcscr")
                            nc.vector.reciprocal_approx_accurate(rc[:], dn[:], scr[:])
                            nc.vector.tensor_mul(AT[h][:, qsl], oacc[:], rc[:])
                        # output projection for this qt's s-chunks (both heads ready)
                        for sc4 in range(QTL // 128):
                            ssl = bass.ds(qt * QTL + sc4 * 128, 128)
                            for et in range(NET):
                                esl = bass.ts(et, ETL)
                                psy = psyp.tile([128, ETL], F32, tag="psy")
                                for h in range(HPC):
                                    nc.tensor.matmul(
                                        psy[:], AT[h][:, ssl], wo_sb[:, h, esl],
                                        start=(h == 0), stop=(h == HPC - 1),
                                    )
                                yt = yp.tile([128, ETL], F32, tag="yt")
                                if et % 2 == 0:
                                    nc.vector.tensor_copy(yt[:], psy[:])
                                else:
                                    nc.scalar.activation(yt[:], psy[:],
                                                         mybir.ActivationFunctionType.Copy)
                                nc.sync.dma_start(y_d[ssl, esl], yt[:])

    nc.compile()
    return nc


def _host_prep(x, w_qkv, b_qkv, w_out, S):
    """Build per-core input maps."""
    xT = np.ascontiguousarray(x.reshape(S, D).T).astype(np.float32)

    # RoPE tables (match reference._rope_cos_sin)
    inv_freq = (1.0 / (10000.0 ** (np.arange(0, ROT, 2, dtype=np.float32) / ROT))).astype(np.float32)
    t = np.arange(S, dtype=np.float32)
    freqs = np.outer(t, inv_freq)                      # [S, ROT/2]
    emb = np.concatenate([freqs, freqs], axis=-1)      # [S, ROT]
    cosT = np.cos(emb).astype(np.float32).T            # [ROT, S]
    sinT = np.sin(emb).astype(np.float32).T
    sinS = sinT.copy()
    sinS[0 : ROT // 2] *= -1.0
    cs = np.ascontiguousarray(np.stack([cosT, sinS], axis=1))  # [ROT, 2, S]

    in_maps = []
    for c in range(NCORES):
        cols = []
        bcols = []
        for h in [HPC * c + i for i in range(HPC)]:
            for part in range(3):  # q, k, v
                off = part * D + h * HD
                cols.append(w_qkv[:, off : off + HD])
                bcols.append(b_qkv[off : off + HD])
        wsl = np.ascontiguousarray(np.stack(cols, axis=1)).astype(np.float32)   # [D, 3*HPC, 128]
        bsl = np.ascontiguousarray(np.stack(bcols, axis=1)).astype(np.float32)  # [128, 3*HPC]
        wout_sl = np.ascontiguousarray(w_out[c * HPC * HD : (c + 1) * HPC * HD, :]).astype(np.float32)
        in_maps.append({"xT": xT, "wsl": wsl, "bsl": bsl, "wout": wout_sl, "cs": cs})
    return in_maps


def kernel(x, w_qkv, b_qkv, w_out, b_out):
    B, S, D_ = x.shape
    assert B == 1 and D_ == D
    if "nc" not in _CACHE:
        _CACHE["nc"] = build_module(S=S)
    nc = _CACHE["nc"]
    in_maps = _host_prep(np.asarray(x, dtype=np.float32), np.asarray(w_qkv, dtype=np.float32),
                         np.asarray(b_qkv, dtype=np.float32), np.asarray(w_out, dtype=np.float32), S)
    res = run_bass_kernel_spmd(nc, in_maps, list(range(NCORES)))
    y = np.zeros((S, D), dtype=np.float32)
    for c in range(NCORES):
        y += res.results[c]["y"]
    y += np.asarray(b_out, dtype=np.float32)[None, :]
    return y.reshape(1, S, D)


# revision 9
# speedup vs baseline: 1.2808x; 1.1731x over previous
"""Multi-head self-attention (B=1, S=4096, D=2048, H=16, rotary_dim=64) on 8 TRN2 NeuronCores.

Head-sharded tensor parallelism: each core computes 2 heads end-to-end
(QKV projection + RoPE + full softmax attention) plus its slice of the
row-sharded output projection; the 8 partial [S, D] outputs are summed on
the host.

Precision: fp16 operands for all matmuls (PE accumulates in fp32; fp16
mantissa ~ matches the f32r/TF32 precision the PE offers for 4-byte
inputs, but gets fast 2-byte weight loads and DVE 2x modes). Softmax
skips max-subtraction (scores are ~N(0,1); exp is safe), computing
exp(s)/sum(exp(s)) directly with an fp32 PSUM denominator reduction.
"""

import numpy as np

import concourse.bass as bass
import concourse.mybir as mybir
import concourse.tile as tile
from concourse import bacc
from concourse.bass_utils import run_bass_kernel_spmd
from concourse.masks import make_identity

F32 = mybir.dt.float32
F32R = mybir.dt.float32r
FP16 = mybir.dt.float16
BF16 = mybir.dt.bfloat16

D = 2048
H = 16
HD = 128
ROT = 64
NCORES = 8
HPC = H // NCORES  # heads per core
SCALE = float(HD) ** -0.5

_CACHE = {}


def build_module(S=4096, ST=512, QTL=512):
    """Build the per-core SPMD bass module. Returns compiled nc."""
    NST = S // ST        # phase-1 s-tiles
    NKT = D // 128       # contraction tiles for QKV
    NQT = S // QTL       # phase-2 q-tiles
    NKC = S // 128       # attention k-chunks
    NKP = NKC // 2       # attention k-chunk pairs
    NSC = S // 128       # out-proj s-chunks
    ETL = 512            # out-proj e-tile
    NET = D // ETL

    nc = bacc.Bacc(None, target_bir_lowering=False, debug=True)

    xT_d = nc.dram_tensor("xT", [D, S], FP16, kind="ExternalInput")
    w_d = nc.dram_tensor("wsl", [D, 3 * HPC, 128], FP16, kind="ExternalInput")
    wo_d = nc.dram_tensor("wout", [HPC * HD, D], FP16, kind="ExternalInput")
    b_d = nc.dram_tensor("bsl", [128, 3 * HPC], F32, kind="ExternalInput")
    cs_d = nc.dram_tensor("cs", [ROT, 2, S], FP16, kind="ExternalInput")
    y_d = nc.dram_tensor("y", [S, D], F32, kind="ExternalOutput")

    xT_r = xT_d[:].rearrange("(t p) s -> p t s", p=128)
    w_r = w_d[:].rearrange("(t p) j m -> p t j m", p=128)
    wo_r = wo_d[:].rearrange("(t p) e -> p t e", p=128)

    with tile.TileContext(nc) as tc:
        with tc.tile_pool(name="persist", bufs=1) as P:
            # Persistent per-head tensors
            QT = [P.tile([128, S], FP16, tag=f"qt{h}", name=f"qt{h}") for h in range(HPC)]
            KT = [P.tile([128, S], FP16, tag=f"kt{h}", name=f"kt{h}") for h in range(HPC)]
            V = [P.tile([128, NKC, 128], FP16, tag=f"v{h}", name=f"v{h}") for h in range(HPC)]
            b_sb = P.tile([128, 3 * HPC], F32)
            identr = P.tile([128, 128], FP16)
            ones_r = P.tile([128, 128], FP16)
            nc.sync.dma_start(b_sb[:], b_d[:])
            make_identity(nc, identr)
            nc.vector.memset(ones_r[:], 1.0)

            # ---------- Phase 1: QKV projection + RoPE + V transpose ----------
            with (
                tc.tile_pool(name="wp", bufs=1) as wp,
                tc.tile_pool(name="xp", bufs=2) as xp,
                tc.tile_pool(name="csp", bufs=2) as csp,
                tc.tile_pool(name="vtp", bufs=2) as vtp,
                tc.tile_pool(name="rtp", bufs=2) as rtp,
                tc.tile_pool(name="ps1", bufs=1, space="PSUM") as ps1,
                tc.tile_pool(name="pst", bufs=2, space="PSUM") as pst,
            ):
                w_sb = wp.tile([128, NKT, 3 * HPC, 128], FP16)
                nc.sync.dma_start(w_sb[:, 0 : NKT // 2, :, :], w_r[:, 0 : NKT // 2, :, :])
                nc.sync.dma_start(w_sb[:, NKT // 2 :, :, :], w_r[:, NKT // 2 :, :, :])

                KH = NKT // 2
                for st in range(NST):
                    sl = bass.ts(st, ST)
                    cst = csp.tile([ROT, 2, ST], FP16, tag="cst")
                    nc.sync.dma_start(cst[:], cs_d[:, :, sl])

                    pss_qkv = {}
                    for h in range(HPC):
                        pss_qkv[(h, 0)] = ps1.tile([128, ST], F32, tag=f"psq{h}", name=f"psq{h}_{st}")
                        pss_qkv[(h, 1)] = ps1.tile([128, ST], F32, tag=f"psk{h}", name=f"psk{h}_{st}")
                        pss_qkv[(h, 2)] = ps1.tile([128, ST], F32, tag=f"psv{h}", name=f"psv{h}_{st}")
                    for kh in range(2):
                        xt = xp.tile([128, KH, ST], FP16, tag="xt")
                        nc.sync.dma_start(xt[:], xT_r[:, kh * KH : (kh + 1) * KH, sl])
                        for h in range(HPC):
                            j = 3 * h
                            for which in range(3):
                                ps = pss_qkv[(h, which)]
                                for k in range(KH):
                                    nc.tensor.matmul(
                                        ps[:], w_sb[:, kh * KH + k, j + which, :], xt[:, k, :],
                                        start=(kh == 0 and k == 0), stop=(kh == 1 and k == KH - 1),
                                    )
                    for h in range(HPC):
                        j = 3 * h
                        for which, dst in ((0, QT[h]), (1, KT[h])):
                            nc.scalar.activation(
                                dst[:, sl], pss_qkv[(h, which)][:],
                                mybir.ActivationFunctionType.Identity,
                                bias=b_sb[:, j + which : j + which + 1],
                            )
                        vt = vtp.tile([128, ST], FP16, tag="vt")
                        nc.scalar.activation(
                            vt[:], pss_qkv[(h, 2)][:], mybir.ActivationFunctionType.Identity,
                            bias=b_sb[:, j + 2 : j + 3],
                        )
                        for sc in range(ST // 128):
                            ptr = pst.tile([128, 128], FP16, tag="ptr")
                            nc.tensor.transpose(ptr[:], vt[:, bass.ts(sc, 128)], identr[:])
                            nc.scalar.activation(V[h][:, st * (ST // 128) + sc, :], ptr[:],
                                                 mybir.ActivationFunctionType.Copy)

                        # RoPE on rotary partitions of this s-slice
                        for T in (QT[h], KT[h]):
                            tmp = rtp.tile([ROT, ST], FP16, tag="rtmp")
                            nc.vector.tensor_copy(tmp[0 : ROT // 2, :], T[ROT // 2 : ROT, sl])
                            nc.vector.tensor_copy(tmp[ROT // 2 : ROT, :], T[0 : ROT // 2, sl])
                            nc.vector.tensor_mul(tmp[:], tmp[:], cst[:, 1, :])
                            nc.vector.tensor_mul(T[0:ROT, sl], T[0:ROT, sl], cst[:, 0, :])
                            nc.vector.tensor_add(T[0:ROT, sl], T[0:ROT, sl], tmp[:])

            # ---------- Phase 2+3: attention with folded output projection ----------
            with tc.tile_pool(name="atp", bufs=1) as atp:
                AT = [atp.tile([128, S], FP16, tag=f"at{h}", name=f"at{h}") for h in range(HPC)]
                with (
                    tc.tile_pool(name="wop", bufs=1) as wop,
                    tc.tile_pool(name="ptp", bufs=4) as ptp,
                    tc.tile_pool(name="accAp", bufs=2) as accAp,
                    tc.tile_pool(name="accBp", bufs=2) as accBp,
                    tc.tile_pool(name="rcp", bufs=2) as rcp,
                    tc.tile_pool(name="yp", bufs=3) as yp,
                    tc.tile_pool(name="pss", bufs=3, space="PSUM") as pssp,
                    tc.tile_pool(name="pso", bufs=2, space="PSUM") as psop,
                    tc.tile_pool(name="psd", bufs=1, space="PSUM") as psdp,
                    tc.tile_pool(name="psy", bufs=2, space="PSUM") as psyp,
                ):
                    wo_sb = wop.tile([128, HPC, D], FP16)
                    nc.sync.dma_start(wo_sb[:], wo_r)
                    LAG = 3
                    for qt in range(NQT):
                        qsl = bass.ts(qt, QTL)
                        for h in range(HPC):
                            oacc = psop.tile([128, QTL], F32, tag="oacc")
                            accA = accAp.tile([128, 2, QTL], FP16, tag="accA")
                            accB = accBp.tile([128, 2, QTL], FP16, tag="accB")
                            pts = {}
                            for kc in range(NKC + LAG):
                                if kc < NKC:
                                    # scores + exp for chunk kc
                                    pss = pssp.tile([128, QTL], F32, tag="pss",
                                                    name=f"pss_{qt}_{h}_{kc}")
                                    nc.tensor.matmul(
                                        pss[:], KT[h][:, bass.ts(kc, 128)], QT[h][:, qsl],
                                        start=True, stop=True,
                                    )
                                    kp = kc // 2
                                    if kc % 2 == 0:
                                        pts[kp] = ptp.tile([128, 2, QTL], FP16, tag="pt",
                                                           name=f"pt_{qt}_{h}_{kp}")
                                    nc.scalar.activation(
                                        pts[kp][:, kc % 2, :], pss[:],
                                        mybir.ActivationFunctionType.Exp, scale=SCALE,
                                    )
                                kd = kc - LAG
                                if 0 <= kd < NKC:
                                    # AV accumulate for chunk kd (lagged for pipelining)
                                    kp = kd // 2
                                    nc.tensor.matmul(
                                        oacc[:], V[h][:, kd, :], pts[kp][:, kd % 2, :],
                                        start=(kd == 0), stop=(kd == NKC - 1),
                                    )
                                    if kd % 2 == 1:
                                        # denominator partial sums, split DVE / GpSimd (~2:1)
                                        if kp == 0:
                                            nc.vector.tensor_copy(accA[:], pts[kp][:])
                                        elif kp == 1:
                                            nc.gpsimd.tensor_copy(accB[:], pts[kp][:])
                                        elif kp % 3 == 1:
                                            nc.gpsimd.tensor_add(accB[:], accB[:], pts[kp][:])
                                        else:
                                            nc.vector.tensor_add(accA[:], accA[:], pts[kp][:])
                                        del pts[kp]
                            # denominator: all-ones matmul -> broadcast across partitions
                            dn = psdp.tile([128, QTL], F32, tag="dn")
                            nc.tensor.matmul(dn[:], ones_r[:], accA[:, 0, :], start=True, stop=False)
                            nc.tensor.matmul(dn[:], ones_r[:], accA[:, 1, :], start=False, stop=False)
                            nc.tensor.matmul(dn[:], ones_r[:], accB[:, 0, :], start=False, stop=False)
                            nc.tensor.matmul(dn[:], ones_r[:], accB[:, 1, :], start=False, stop=True)
                            rc = rcp.tile([128, QTL], F32, tag="rc")
                            scr = rcp.tile([128, QTL], F32, tag="rcscr")
                            nc.vector.reciprocal_approx_accurate(rc[:], dn[:], scr[:])
                            nc.vector.tensor_mul(AT[h][:, qsl], oacc[:], rc[:])
                        # output projection for this qt's s-chunks (both heads ready)
                        for sc4 in range(QTL // 128):
                            ssl = bass.ds(qt * QTL + sc4 * 128, 128)
                            for et in range(NET):
                                esl = bass.ts(et, ETL)
                                psy = psyp.tile([128, ETL], F32, tag="psy")
                                for h in range(HPC):
                                    nc.tensor.matmul(
                                        psy[:], AT[h][:, ssl], wo_sb[:, h, esl],
                                        start=(h == 0), stop=(h == HPC - 1),
                                    )
                                yt = yp.tile([128, ETL], F32, tag="yt")
                                if et % 2 == 0:
                                    nc.vector.tensor_copy(yt[:], psy[:])
                                else:
                                    nc.scalar.activation(yt[:], psy[:],
                                                         mybir.ActivationFunctionType.Copy)
                                nc.sync.dma_start(y_d[ssl, esl], yt[:])

    nc.compile()
    return nc


def _host_prep(x, w_qkv, b_qkv, w_out, S):
    """Build per-core input maps."""
    xT = np.ascontiguousarray(x.reshape(S, D).T).astype(np.float16)

    # RoPE tables (match reference._rope_cos_sin)
    inv_freq = (1.0 / (10000.0 ** (np.arange(0, ROT, 2, dtype=np.float32) / ROT))).astype(np.float32)
    t = np.arange(S, dtype=np.float32)
    freqs = np.outer(t, inv_freq)                      # [S, ROT/2]
    emb = np.concatenate([freqs, freqs], axis=-1)      # [S, ROT]
    cosT = np.cos(emb).astype(np.float32).T            # [ROT, S]
    sinT = np.sin(emb).astype(np.float32).T
    sinS = sinT.copy()
    sinS[0 : ROT // 2] *= -1.0
    cs = np.ascontiguousarray(np.stack([cosT, sinS], axis=1)).astype(np.float16)  # [ROT, 2, S]

    in_maps = []
    for c in range(NCORES):
        cols = []
        bcols = []
        for h in [HPC * c + i for i in range(HPC)]:
            for part in range(3):  # q, k, v
                off = part * D + h * HD
                cols.append(w_qkv[:, off : off + HD])
                bcols.append(b_qkv[off : off + HD])
        wsl = np.ascontiguousarray(np.stack(cols, axis=1)).astype(np.float16)   # [D, 3*HPC, 128]
        bsl = np.ascontiguousarray(np.stack(bcols, axis=1)).astype(np.float32)  # [128, 3*HPC]
        wout_sl = np.ascontiguousarray(w_out[c * HPC * HD : (c + 1) * HPC * HD, :]).astype(np.float16)
        in_maps.append({"xT": xT, "wsl": wsl, "bsl": bsl, "wout": wout_sl, "cs": cs})
    return in_maps


def kernel(x, w_qkv, b_qkv, w_out, b_out):
    B, S, D_ = x.shape
    assert B == 1 and D_ == D
    if "nc" not in _CACHE:
        _CACHE["nc"] = build_module(S=S)
    nc = _CACHE["nc"]
    in_maps = _host_prep(np.asarray(x, dtype=np.float32), np.asarray(w_qkv, dtype=np.float32),
                         np.asarray(b_qkv, dtype=np.float32), np.asarray(w_out, dtype=np.float32), S)
    res = run_bass_kernel_spmd(nc, in_maps, list(range(NCORES)))
    y = np.zeros((S, D), dtype=np.float32)
    for c in range(NCORES):
        y += res.results[c]["y"]
    y += np.asarray(b_out, dtype=np.float32)[None, :]
    return y.reshape(1, S, D)
